# revision 1
# baseline (speedup 1.0000x reference)
"""AttentionBlock3D (GroupNorm + single-head self-attention + residual) on 8 TRN2 cores.

Sharding: core = (batch b in {0,1}) x (1024-row slice of the 4096 attention rows).
Each core redundantly computes its batch's GroupNorm stats and full K/V
(cheap), and attention + output projection for its own 1024 query rows.
No collectives. The host ROTATES each core's x copy so that its query rows
are always columns 0..1024 (attention is permutation-invariant over keys).

Math folding (all computed on-device from the real inputs; nothing assumes
zero biases):
  hn = x*A + B per channel, with A = gamma*rsqrt(var_g+eps), B = beta - mean_g*A
  q  = (Wq . A) x_q + cq           cq = Wq B + bq   (bias folded into q copy)
  k  = (Wk . A) x                  (k bias cancels in softmax over keys)
  v0 = (Wv . A) x                  cv = Wv B + bv   (rows of softmax sum to 1
                                   => P @ (cv 1^T) = cv 1^T, folded into bpe)
  S^T[m,n] = sum_o k[o,m] q[o,n];  E = exp(S/16);  r[n] = sum_m E[m,n]
  out = (x_q + bpe) + ((Wp^T)^T @ (E^T V)) * (1/r),  bpe = bp + Wp cv
"""

import os
import numpy as np
from contextlib import ExitStack

C = 256          # channels
N = 4096         # spatial positions (16*16*16)
NQ = 1024        # query rows per core
GROUPS = 8
GSIZE = C // GROUPS
EPS = 1e-5
NCH = NQ // 512  # n-chunks per core

_CACHE = {}
LAST_RESULTS = None  # test harness can inspect trace results


def _build_nc(use_f32r=True):
    import concourse.bacc as bacc
    import concourse.bass as bass
    import concourse.tile as tile
    from concourse import mybir

    f32 = mybir.dt.float32
    f32r = mybir.dt.float32r
    AF = mybir.ActivationFunctionType

    fr = f32r if use_f32r else f32
    xbf16 = os.environ.get("BASSK_XBF16", "1") == "1"
    bx = mybir.dt.bfloat16 if xbf16 else fr

    def R(ap):
        return ap

    nc = bacc.Bacc("TRN2", target_bir_lowering=False, debug=False,
                   enable_asserts=False)

    # ---- DRAM I/O (per-core) ----
    xb_d = nc.dram_tensor("xb", [C, N],
                          mybir.dt.bfloat16 if os.environ.get("BASSK_XBF16", "1") == "1" else f32,
                          kind="ExternalInput").ap()
    xq_d = nc.dram_tensor("xq", [C, NQ], f32, kind="ExternalInput").ap()
    wall_d = nc.dram_tensor("wall", [C, 4 * C], f32, kind="ExternalInput").ap()
    small_d = nc.dram_tensor("small", [C, 5 + GROUPS], f32, kind="ExternalInput").ap()
    gmask8_d = nc.dram_tensor("gmask8", [GROUPS, C], f32, kind="ExternalInput").ap()
    out_d = nc.dram_tensor("out", [C, NQ], f32, kind="ExternalOutput").ap()

    with tile.TileContext(nc) as tc, ExitStack() as ctx:
        big = ctx.enter_context(tc.tile_pool(name="big", bufs=1))
        consts = ctx.enter_context(tc.tile_pool(name="consts", bufs=1))
        work = ctx.enter_context(tc.tile_pool(name="work", bufs=3))
        pw = ctx.enter_context(tc.tile_pool(name="pw", bufs=3, space="PSUM"))
        pacc = ctx.enter_context(tc.tile_pool(name="pacc", bufs=3, space="PSUM"))
        pr = ctx.enter_context(tc.tile_pool(name="pr", bufs=1, space="PSUM"))
        pstat = ctx.enter_context(tc.tile_pool(name="pstat", bufs=1, space="PSUM"))

        # ---- constants / small loads (before the big x load) ----
        ones_f32 = consts.tile([128, 128], f32)
        nc.vector.memset(ones_f32, 1.0)
        ones128 = consts.tile([128, 128], fr)
        nc.vector.tensor_copy(ones128, ones_f32)
        # eps8 = Sqrt(EPS^2) on ACT: forces the Sqrt act-table load to run at
        # t~0 (gsd depends on eps8, so the scheduler cannot sink it)
        eps_sq = consts.tile([GROUPS, 1], f32)
        nc.vector.memset(eps_sq, EPS * EPS)
        eps8 = consts.tile([GROUPS, 1], f32)
        nc.scalar.activation(out=eps8, in_=eps_sq, func=AF.Sqrt, scale=1.0)

        # ---- load x first (chunked, stats interleaved), then consts/weights ----
        xb_sb = []
        stats_l = []
        for ct in range(2):
            cs = slice(ct * 128, (ct + 1) * 128)
            t = big.tile([128, N], bx, name=f"xb_sb{ct}")
            stats = work.tile([128, 8, 6], f32, name="stats", tag="stats")
            for s in range(2):
                fs = slice(s * 2048, (s + 1) * 2048)
                nc.sync.dma_start(out=t[:, fs],
                                  in_=xb_d[cs, fs] if xbf16 else xb_d[cs, fs].bitcast(fr))
                for s2 in range(4):
                    ss = slice(s * 2048 + s2 * 512, s * 2048 + (s2 + 1) * 512)
                    nc.vector.bn_stats(out=stats[:, s * 4 + s2, :],
                                       in_=t[:, ss] if xbf16 else t[:, ss].bitcast(f32))
            xb_sb.append(t)
            stats_l.append(stats)
        xq = []
        for ct in range(2):
            cs = slice(ct * 128, (ct + 1) * 128)
            t = big.tile([128, NQ], f32, name=f"xq_sb{ct}")
            nc.sync.dma_start(out=t, in_=xq_d[cs, :])
            xq.append(t)

        small_sb, wall_sb = [], []
        for ct in range(2):
            cs = slice(ct * 128, (ct + 1) * 128)
            t = consts.tile([128, 5 + GROUPS], f32, name=f"small_sb{ct}")
            nc.sync.dma_start(out=t, in_=small_d[cs, :]); small_sb.append(t)
        gmask8_sb = consts.tile([GROUPS, C], f32)
        nc.sync.dma_start(out=gmask8_sb, in_=gmask8_d)
        for ct in range(2):
            cs = slice(ct * 128, (ct + 1) * 128)
            t = consts.tile([128, 4 * C], f32, name=f"wall_sb{ct}")
            nc.sync.dma_start(out=t, in_=wall_d[cs, :]); wall_sb.append(t)
        gamma_sb = [t[:, 0:1] for t in small_sb]
        beta_sb = [t[:, 1:2] for t in small_sb]
        bq_sb = [t[:, 2:3] for t in small_sb]
        bv_sb = [t[:, 3:4] for t in small_sb]
        bp_sb = [t[:, 4:5] for t in small_sb]
        gmask_sb = [t[:, 5:5 + GROUPS] for t in small_sb]
        wqt_sb = [t[:, 0 * C:1 * C] for t in wall_sb]
        wkt_sb = [t[:, 1 * C:2 * C] for t in wall_sb]
        wvt_sb = [t[:, 2 * C:3 * C] for t in wall_sb]
        wpt_sb = [t[:, 3 * C:4 * C] for t in wall_sb]

        # per-channel moments -> group sums via 0/1 mask matmul (exact fp32)
        gp = pstat.tile([GROUPS, 2], f32, tag="pstat")
        for ct in range(2):
            stile = work.tile([128, 2], f32, name="stile", tag="stile")
            msq = work.tile([128, 1], f32, name="msq", tag="msq")
            nc.vector.bn_aggr(out=stile, in_=stats_l[ct])
            nc.vector.tensor_mul(msq, stile[:, 0:1], stile[:, 0:1])
            nc.vector.tensor_add(stile[:, 1:2], stile[:, 1:2], msq)
            nc.tensor.matmul(gp, lhsT=gmask_sb[ct], rhs=stile,
                             start=(ct == 0), stop=(ct == 1))

        # ---- group stats -> per-channel A, B (PE mask8 broadcast, no DMA) ----
        gms = work.tile([GROUPS, 2], f32, name="gms")
        gvar = work.tile([GROUPS, 1], f32, name="gvar")
        gsd = work.tile([GROUPS, 1], f32, name="gsd")
        gsb = work.tile([GROUPS, 2], f32, name="gsb")
        nc.vector.tensor_scalar_mul(gms, gp, 1.0 / GSIZE)
        nc.vector.tensor_mul(gvar, gms[:, 0:1], gms[:, 0:1])
        nc.vector.tensor_sub(gvar, gms[:, 1:2], gvar)
        nc.scalar.activation(out=gsd, in_=gvar, func=AF.Sqrt, bias=eps8, scale=1.0)
        nc.vector.tensor_copy(gsb[:, 0:1], gms[:, 0:1])
        nc.vector.reciprocal(out=gsb[:, 1:2], in_=gsd)

        A_sb, B_sb = [], []
        for ct in range(2):
            gbp = pstat.tile([128, 2], f32, name="gbp", tag="pstat")
            nc.tensor.matmul(gbp, lhsT=gmask8_sb[:, ct * 128:(ct + 1) * 128],
                             rhs=gsb, start=True, stop=True)
            At = consts.tile([128, 1], f32, name=f"A_sb{ct}")
            Bt = consts.tile([128, 1], f32, name=f"B_sb{ct}")
            nc.vector.tensor_mul(At, gamma_sb[ct], gbp[:, 1:2])
            nc.vector.tensor_mul(Bt, gbp[:, 0:1], At)
            nc.vector.tensor_sub(Bt, beta_sb[ct], Bt)
            A_sb.append(At); B_sb.append(Bt)

        # ---- fold A into weights (split across DVE and ACT) ----
        wqte, wkte, wvte = [], [], []
        for lst, wsrc, nm in ((wkte, wkt_sb, "wkte"), (wqte, wqt_sb, "wqte"),
                              (wvte, wvt_sb, "wvte")):
            for ct in range(2):
                t = consts.tile([128, C], bx, name=f"{nm}{ct}")
                if ct == 0:
                    nc.vector.tensor_scalar_mul(t, wsrc[ct], A_sb[ct])
                else:
                    nc.scalar.activation(out=t, in_=wsrc[ct], func=AF.Identity,
                                         scale=A_sb[ct])
                lst.append(t)
        # preload the Exp act table while PE/DVE are busy with k/q/v
        dummy3 = consts.tile([1, 1], f32)
        nc.scalar.activation(out=dummy3, in_=A_sb[1][0:1, :], func=AF.Exp, scale=1.0)

        # ---- k = (Wk.A) x   [o, m] layout (first PE bulk work) ----
        k_sb = [big.tile([128, N], fr, name=f"k_sb{ot}") for ot in range(2)]
        q_sb = [big.tile([128, NQ], fr, name=f"q_sb{ot}") for ot in range(2)]
        for ot in range(2):
            os_ = slice(ot * 128, (ot + 1) * 128)
            for mc in range(8):
                fs = slice(mc * 512, (mc + 1) * 512)
                kp = pw.tile([128, 512], f32, name="kp", tag="pw")
                for ct in range(2):
                    nc.tensor.matmul(kp, lhsT=R(wkte[ct][:, os_]),
                                     rhs=R(xb_sb[ct][:, fs]),
                                     start=(ct == 0), stop=(ct == 1))
                if mc % 2 == 0:
                    nc.vector.tensor_copy(k_sb[ot][:, fs], kp)
                else:
                    nc.scalar.copy(k_sb[ot][:, fs], kp)

        # ---- bias vectors (PE cost tiny; overlaps with k copies) ----
        def bias_vec(wt_sb, rhs_tiles, badd, nm):
            outs = []
            for oh in range(2):
                p = pstat.tile([128, 1], f32, name=f"{nm}p", tag="pstat")
                for ct in range(2):
                    nc.tensor.matmul(p, lhsT=wt_sb[ct][:, oh * 128:(oh + 1) * 128],
                                     rhs=rhs_tiles[ct], start=(ct == 0), stop=(ct == 1))
                t = consts.tile([128, 1], f32, name=f"{nm}{oh}")
                nc.scalar.activation(out=t, in_=p, func=AF.Identity,
                                     bias=badd[oh], scale=1.0)
                outs.append(t)
            return outs

        cq_sb = bias_vec(wqt_sb, B_sb, bq_sb, "cq")
        cv_sb = bias_vec(wvt_sb, B_sb, bv_sb, "cv")
        bpe_sb = bias_vec(wpt_sb, cv_sb, bp_sb, "bpe")

        # f32r copy of Wp^T for the projection matmuls
        wpte = []
        for ct in range(2):
            t = consts.tile([128, C], fr, name=f"wpte{ct}")
            nc.vector.tensor_copy(t, wpt_sb[ct])
            wpte.append(t)

        # residual+bias base: xqb = x_q + bpe (off critical path)
        xqb = [big.tile([128, NQ], f32, name=f"xqb{ot}") for ot in range(2)]
        for ot in range(2):
            nc.gpsimd.tensor_scalar_add(xqb[ot], xq[ot], bpe_sb[ot])

        # ---- q = (Wq.A) x_q + cq ----
        for ot in range(2):
            os_ = slice(ot * 128, (ot + 1) * 128)
            for qc in range(NCH):
                fs = slice(qc * 512, (qc + 1) * 512)
                qp = pw.tile([128, 512], f32, name="qp", tag="pw")
                for ct in range(2):
                    nc.tensor.matmul(qp, lhsT=R(wqte[ct][:, os_]),
                                     rhs=R(xb_sb[ct][:, fs]),
                                     start=(ct == 0), stop=(ct == 1))
                if ot == 0:
                    nc.scalar.activation(out=q_sb[ot][:, fs], in_=qp,
                                         func=AF.Identity, bias=cq_sb[ot], scale=1.0)
                else:
                    nc.vector.tensor_scalar_add(q_sb[ot][:, fs], qp, cq_sb[ot])

        # ---- v = (Wv.A) x   [m, o] layout ----
        v_sb = big.tile([128, 32, C], fr, name="v_sb")
        for mt in range(32):
            ms = slice(mt * 128, (mt + 1) * 128)
            vp = pw.tile([128, C], f32, name="vp", tag="pw")
            for ct in range(2):
                nc.tensor.matmul(vp, lhsT=R(xb_sb[ct][:, ms]), rhs=R(wvte[ct]),
                                 start=(ct == 0), stop=(ct == 1))
            if mt % 2 == 0:
                nc.scalar.copy(v_sb[:, mt, :], vp)
            else:
                nc.vector.tensor_copy(v_sb[:, mt, :], vp)

        # ---- attention + projection, per n-chunk ----
        att_sb = [big.tile([128, NQ], fr, name=f"att_sb{ot}") for ot in range(2)]
        for nch in range(NCH):
            ns = slice(nch * 512, (nch + 1) * 512)
            otp = [pacc.tile([128, 512], f32, name=f"otp{oh}", tag="acc")
                   for oh in range(2)]
            rp = pr.tile([128, 512], f32, name="rp", tag="pr")
            for mt in range(32):
                ms = slice(mt * 128, (mt + 1) * 128)
                sp = pw.tile([128, 512], f32, name="sp", tag="pw")
                for ot in range(2):
                    nc.tensor.matmul(sp, lhsT=R(k_sb[ot][:, ms]),
                                     rhs=R(q_sb[ot][:, ns]),
                                     start=(ot == 0), stop=(ot == 1))
                e = work.tile([128, 512], fr, name="e", tag="e")
                nc.scalar.activation(out=e, in_=sp, func=AF.Exp, scale=1.0 / 16.0)
                nc.tensor.matmul(rp, lhsT=R(ones128), rhs=R(e),
                                 start=(mt == 0), stop=(mt == 31))
                for oh in range(2):
                    nc.tensor.matmul(otp[oh],
                                     lhsT=R(v_sb[:, mt, oh * 128:(oh + 1) * 128]),
                                     rhs=R(e), start=(mt == 0), stop=(mt == 31))
            rb = work.tile([128, 512], f32, name="rb", tag="rb", bufs=2)
            nc.vector.reciprocal(out=rb, in_=rp)
            # att = (E^T V) * (1/r): normalization folded into the psum drain
            for oh in range(2):
                nc.vector.tensor_mul(att_sb[oh][:, ns], otp[oh], rb)
            # projection + add-only epilogue for this chunk
            last = (nch == NCH - 1)
            st_engines = [nc.sync, nc.scalar] if last else [nc.sync, nc.sync]
            for ot in range(2):
                os_ = slice(ot * 128, (ot + 1) * 128)
                pp = pacc.tile([128, 512], f32, name="pp", tag="acc")
                for ct in range(2):
                    nc.tensor.matmul(pp, lhsT=R(wpte[ct][:, os_]),
                                     rhs=R(att_sb[ct][:, ns]),
                                     start=(ct == 0), stop=(ct == 1))
                ot_t = work.tile([128, 512], f32, name="ot_t", tag="ot_t")
                for hh in range(2):
                    hs = slice(hh * 256, (hh + 1) * 256)
                    ds = slice(nch * 512 + hh * 256, nch * 512 + (hh + 1) * 256)
                    nc.vector.tensor_add(ot_t[:, hs], pp[:, hs], xqb[ot][:, ds])
                st_engines[ot].dma_start(out=out_d[os_, ns], in_=ot_t)

    nc.compile()
    return nc


def _get_nc():
    key = "nc"
    if key not in _CACHE:
        _CACHE[key] = _build_nc(use_f32r=(os.environ.get("BASSK_F32R", "1") == "1"))
    return _CACHE[key]


def _host_inputs(x, gamma, beta, Wq, bq, Wk, bk, Wv, bv, Wp, bp):
    x = np.asarray(x, np.float32)
    xf = np.ascontiguousarray(x.reshape(2, C, N))
    gamma = np.asarray(gamma, np.float32).reshape(C, 1)
    beta = np.asarray(beta, np.float32).reshape(C, 1)
    wqt = np.ascontiguousarray(np.asarray(Wq, np.float32).T)
    wkt = np.ascontiguousarray(np.asarray(Wk, np.float32).T)
    wvt = np.ascontiguousarray(np.asarray(Wv, np.float32).T)
    wpt = np.ascontiguousarray(np.asarray(Wp, np.float32).T)
    bq = np.asarray(bq, np.float32).reshape(C, 1)
    bv = np.asarray(bv, np.float32).reshape(C, 1)
    bp = np.asarray(bp, np.float32).reshape(C, 1)
    gmask = np.zeros((C, GROUPS), np.float32)
    gmask[np.arange(C), np.arange(C) // GSIZE] = 1.0
    gmask8 = np.ascontiguousarray(gmask.T)
    wall = np.ascontiguousarray(np.hstack([wqt, wkt, wvt, wpt]))
    small = np.ascontiguousarray(np.hstack([gamma, beta, bq, bv, bp, gmask]))

    xbf16 = os.environ.get("BASSK_XBF16", "1") == "1"
    if xbf16:
        import ml_dtypes
    in_maps = []
    for core in range(8):
        b, j = divmod(core, 4)
        xrot = np.ascontiguousarray(np.roll(xf[b], -j * NQ, axis=1))
        in_maps.append({
            "xb": xrot.astype(ml_dtypes.bfloat16) if xbf16 else xrot,
            "xq": np.ascontiguousarray(xrot[:, :NQ]),
            "wall": wall, "small": small, "gmask8": gmask8,
        })
    return in_maps


def kernel(x, gamma, beta, Wq, bq, Wk, bk, Wv, bv, Wp, bp):
    from concourse.bass_utils import run_bass_kernel_spmd
    global LAST_RESULTS

    orig_shape = np.asarray(x).shape
    in_maps = _host_inputs(x, gamma, beta, Wq, bq, Wk, bk, Wv, bv, Wp, bp)
    nc = _get_nc()

    trace = os.environ.get("BASSK_TRACE", "0") == "1"
    res = run_bass_kernel_spmd(nc, in_maps, core_ids=list(range(8)), trace=trace)
    LAST_RESULTS = res

    out = np.empty((2, C, N), np.float32)
    for core in range(8):
        b, j = divmod(core, 4)
        out[b][:, j * NQ:(j + 1) * NQ] = res.results[core]["out"]
    return out.reshape(orig_shape)



# revision 9
# speedup vs baseline: 1.8444x; 1.8444x over previous
"""AttentionBlock3D (GroupNorm + single-head self-attention + residual) on 8 TRN2 cores.

Sharding: core = (batch b in {0,1}) x (1024-row slice of the 4096 attention rows).
Each core redundantly computes its batch's GroupNorm stats and full V (cheap),
and attention + output projection for its own 1024 query rows. No collectives.
The host ROTATES each core's x copy so its query rows are always columns
0..1024 (attention is permutation-invariant over keys).

fp8 (e4m3) + DoubleRow PE mode: every 256-deep contraction runs as ONE
matmul over [128, 2, *] operand layouts at 0.5 cycles/row (4x the f32r
baseline). The k/q distinction is folded away:
  hn = A*x + B per channel, A = gamma*rsqrt(var_g+eps), B = beta - mean_g*A
  q  = (16*A.Wq)^T x8 / 16 + cq          cq = Wq B + bq
  t  = A/16 * (16*Wk)^T q8  (== Wk'^T q, the k-side collapsed into t)
  S^T[m,n] = sum_c x8[c,m] t8[c,n]       (k bias cancels in softmax)
  e  = exp(S/16 - 2.5)   (shift cancels in normalization; keeps e in fp8 range)
  r  = ones^T e;  v = (A.Wv)^T x8;  att = 32 * (E^T V) / r
  out = x_q + (bp + Wp cv) + (16Wp^T att)/512,  cv = Wv B + bv
GroupNorm stats come from a stride-4 subsample of the fp8 x copy (32k samples
per group; ~0.4% stat noise, far under tolerance). rsqrt is computed on DVE
with the bit-trick + 2 Newton iterations so ACT only ever needs the Exp table.
"""

import os
import numpy as np
from contextlib import ExitStack

C = 256          # channels
N = 4096         # spatial positions (16*16*16)
NQ = 1024        # query rows per core
GROUPS = 8
GSIZE = C // GROUPS
EPS = 1e-5
ESHIFT = 4.0     # exp(S/16 - ESHIFT): cancels in softmax, keeps e <= ~50 in fp8

_CACHE = {}
LAST_RESULTS = None  # test harness can inspect trace results


def _build_nc():
    import concourse.bacc as bacc
    import concourse.tile as tile
    from concourse import mybir

    f32 = mybir.dt.float32
    fp8 = mybir.dt.float8e4
    i32 = mybir.dt.int32
    AF = mybir.ActivationFunctionType
    AL = mybir.AluOpType
    DR = mybir.MatmulPerfMode.DoubleRow

    nc = bacc.Bacc("TRN2", target_bir_lowering=False, debug=False,
                   enable_asserts=False)

    # ---- DRAM I/O (per-core) ----
    x8_d = nc.dram_tensor("x8", [128, 2 * N], fp8, kind="ExternalInput").ap()
    xq_d = nc.dram_tensor("xq", [C, NQ], f32, kind="ExternalInput").ap()
    wall_d = nc.dram_tensor("wall", [C, 3 * C], f32, kind="ExternalInput").ap()
    wk8_d = nc.dram_tensor("wk8", [128, 2 * C], fp8, kind="ExternalInput").ap()
    wp8_d = nc.dram_tensor("wp8", [128, 2 * C], fp8, kind="ExternalInput").ap()
    small_d = nc.dram_tensor("small", [C, 5 + GROUPS], f32, kind="ExternalInput").ap()
    gmask8_d = nc.dram_tensor("gmask8", [GROUPS, C], f32, kind="ExternalInput").ap()
    out_d = nc.dram_tensor("out", [C, NQ], f32, kind="ExternalOutput").ap()

    with tile.TileContext(nc) as tc, ExitStack() as ctx:
        big = ctx.enter_context(tc.tile_pool(name="big", bufs=1))
        consts = ctx.enter_context(tc.tile_pool(name="consts", bufs=1))
        work = ctx.enter_context(tc.tile_pool(name="work", bufs=4))
        epool = ctx.enter_context(tc.tile_pool(name="epool", bufs=6))
        attp = ctx.enter_context(tc.tile_pool(name="attp", bufs=2))
        # PSUM: psp 2x[128,2,512]=4 banks, pvv 1, pot 2, prp 1 -> 8 banks
        psp = ctx.enter_context(tc.tile_pool(name="psp", bufs=2, space="PSUM"))
        pvv = ctx.enter_context(tc.tile_pool(name="pvv", bufs=1, space="PSUM"))
        pot = ctx.enter_context(tc.tile_pool(name="pot", bufs=1, space="PSUM"))
        prp = ctx.enter_context(tc.tile_pool(name="prp", bufs=1, space="PSUM"))

        # ---- consts + Exp act-table preload at t~0 (ACT's first instr) ----
        dume = consts.tile([1, 2], f32)
        nc.vector.memset(dume, 0.0)
        dume2 = consts.tile([1, 2], f32)
        nc.scalar.activation(out=dume2, in_=dume, func=AF.Exp, scale=1.0)
        ones8 = consts.tile([128, 2, 128], fp8)
        nc.vector.memset(ones8, 1.0)
        magic = consts.tile([GROUPS, 1], i32)
        nc.vector.memset(magic, 0x5F3759DF)
        sh1 = consts.tile([GROUPS, 1], i32)
        nc.vector.memset(sh1, 1)
        eshift = consts.tile([128, 1], f32)
        nc.vector.memset(eshift, -ESHIFT)

        # ---- DMAs (SP program order == priority) ----
        x8 = big.tile([128, 2, N], fp8, name="x8")
        for k in range(4):
            ct, h = divmod(k, 2)
            nc.sync.dma_start(out=x8[:, ct, h * 2048:(h + 1) * 2048],
                              in_=x8_d[:, k * 2048:(k + 1) * 2048])
        wall_sb, small_sb = [], []
        for ct in range(2):
            cs = slice(ct * 128, (ct + 1) * 128)
            t = consts.tile([128, 3 * C], f32, name=f"wall_sb{ct}")
            nc.sync.dma_start(out=t, in_=wall_d[cs, :])
            wall_sb.append(t)
        for ct in range(2):
            cs = slice(ct * 128, (ct + 1) * 128)
            t = consts.tile([128, 5 + GROUPS], f32, name=f"small_sb{ct}")
            nc.sync.dma_start(out=t, in_=small_d[cs, :])
            small_sb.append(t)
        gmask8_sb = consts.tile([GROUPS, C], f32)
        nc.sync.dma_start(out=gmask8_sb, in_=gmask8_d)
        wk8_sb = consts.tile([128, 2, C], fp8)
        nc.sync.dma_start(out=wk8_sb[:, :, :].bitcast(fp8), in_=wk8_d)
        xq_sb = []
        for ot in range(2):
            t = big.tile([128, NQ], f32, name=f"xq{ot}")
            nc.sync.dma_start(out=t, in_=xq_d[ot * 128:(ot + 1) * 128, :])
            xq_sb.append(t)
        wp8_sb = consts.tile([128, 2, C], fp8)
        nc.sync.dma_start(out=wp8_sb[:, :, :].bitcast(fp8), in_=wp8_d)

        gamma_sb = [t[:, 0:1] for t in small_sb]
        beta_sb = [t[:, 1:2] for t in small_sb]
        bq_sb = [t[:, 2:3] for t in small_sb]
        bv_sb = [t[:, 3:4] for t in small_sb]
        bp_sb = [t[:, 4:5] for t in small_sb]
        gmask_sb = [t[:, 5:5 + GROUPS] for t in small_sb]
        wqt_sb = [t[:, 0 * C:1 * C] for t in wall_sb]
        wvt_sb = [t[:, 1 * C:2 * C] for t in wall_sb]
        wpt_sb = [t[:, 2 * C:3 * C] for t in wall_sb]

        # ---- GroupNorm stats from stride-4 subsample (DVE) ----
        gp = prp.tile([GROUPS, 2], f32, tag="prp")
        for ct in range(2):
            stats = work.tile([128, 2, 6], f32, name="stats", tag="stats")
            for h in range(2):
                nc.vector.bn_stats(out=stats[:, h, :],
                                   in_=x8[:, ct, h * 2048:(h + 1) * 2048:4])
            stile = work.tile([128, 2], f32, name="stile", tag="stile")
            msq = work.tile([128, 1], f32, name="msq", tag="msq")
            nc.vector.bn_aggr(out=stile, in_=stats)
            nc.vector.tensor_mul(msq, stile[:, 0:1], stile[:, 0:1])
            nc.vector.tensor_add(stile[:, 1:2], stile[:, 1:2], msq)
            nc.tensor.matmul(gp, lhsT=gmask_sb[ct], rhs=stile,
                             start=(ct == 0), stop=(ct == 1))

        # group mean / E[x^2] -> var -> rsqrt via bit-trick + 2 Newton iters
        gms = work.tile([GROUPS, 2], f32, name="gms")
        gv = work.tile([GROUPS, 1], f32, name="gv")
        gvh = work.tile([GROUPS, 1], f32, name="gvh")
        gi = work.tile([GROUPS, 1], i32, name="gi")
        gy = work.tile([GROUPS, 1], f32, name="gy")
        nt = work.tile([GROUPS, 1], f32, name="nt")
        gsb = work.tile([GROUPS, 2], f32, name="gsb")
        nc.vector.tensor_scalar_mul(gms, gp, 1.0 / GSIZE)
        nc.vector.tensor_mul(gv, gms[:, 0:1], gms[:, 0:1])
        nc.vector.tensor_sub(gv, gms[:, 1:2], gv)
        nc.vector.tensor_scalar_add(gv, gv, EPS)
        nc.vector.tensor_scalar_mul(gvh, gv, 0.5)
        nc.vector.tensor_scalar(out=gi, in0=gv.bitcast(i32), scalar1=sh1,
                                scalar2=None, op0=AL.logical_shift_right)
        nc.vector.tensor_sub(gy.bitcast(i32), magic, gi)
        for _ in range(2):
            nc.vector.tensor_mul(nt, gy, gy)
            nc.vector.tensor_mul(nt, nt, gvh)
            nc.vector.tensor_scalar(out=nt, in0=nt, scalar1=-1.0, scalar2=1.5,
                                    op0=AL.mult, op1=AL.add)
            nc.vector.tensor_mul(gy, gy, nt)
        nc.vector.tensor_copy(gsb[:, 0:1], gms[:, 0:1])
        nc.vector.tensor_copy(gsb[:, 1:2], gy)

        # broadcast group stats to channels; A, B, A*16, A/16
        A_sb, A16_sb, Ai16_sb, B_sb = [], [], [], []
        for ct in range(2):
            gbp = prp.tile([128, 2], f32, name="gbp", tag="prp")
            nc.tensor.matmul(gbp, lhsT=gmask8_sb[:, ct * 128:(ct + 1) * 128],
                             rhs=gsb, start=True, stop=True)
            At = consts.tile([128, 1], f32, name=f"A_sb{ct}")
            A16 = consts.tile([128, 1], f32, name=f"A16_sb{ct}")
            Ai16 = consts.tile([128, 1], f32, name=f"Ai16_sb{ct}")
            Bt = consts.tile([128, 1], f32, name=f"B_sb{ct}")
            nc.vector.tensor_mul(At, gamma_sb[ct], gbp[:, 1:2])
            nc.vector.tensor_mul(Bt, gbp[:, 0:1], At)
            nc.vector.tensor_sub(Bt, beta_sb[ct], Bt)
            nc.vector.tensor_scalar_mul(A16, At, 16.0)
            nc.vector.tensor_scalar_mul(Ai16, At, 1.0 / 16.0)
            A_sb.append(At); A16_sb.append(A16)
            Ai16_sb.append(Ai16); B_sb.append(Bt)

        # fold 16*A into Wq / Wv, quantize to fp8 (DVE)
        wq8 = consts.tile([128, 2, C], fp8, name="wq8")
        wv8 = consts.tile([128, 2, C], fp8, name="wv8")
        for ct in range(2):
            nc.vector.tensor_scalar_mul(wq8[:, ct, :], wqt_sb[ct], A16_sb[ct])
        for ct in range(2):
            nc.vector.tensor_scalar_mul(wv8[:, ct, :], wvt_sb[ct], A16_sb[ct])

        # ---- bias vectors cq, cv, bpe (tiny f32 PE matmuls) ----
        def bias_vec(wt_sb, rhs_tiles, badd, nm):
            outs = []
            for oh in range(2):
                p = prp.tile([128, 1], f32, name=f"{nm}p", tag="prp")
                for ct in range(2):
                    nc.tensor.matmul(p, lhsT=wt_sb[ct][:, oh * 128:(oh + 1) * 128],
                                     rhs=rhs_tiles[ct], start=(ct == 0), stop=(ct == 1))
                t = consts.tile([128, 1], f32, name=f"{nm}{oh}")
                nc.vector.tensor_scalar_add(t, p, badd[oh])
                outs.append(t)
            return outs

        cq_sb = bias_vec(wqt_sb, B_sb, bq_sb, "cq")

        # ---- q = (16A.Wq)^T x8 / 16 + cq   [o, n] fp8, oh on dim1 ----
        q8 = big.tile([128, 2, NQ], fp8, name="q8")
        for qc in range(2):
            qs = slice(qc * 512, (qc + 1) * 512)
            qp = psp.tile([128, 2, 512], f32, name="qp", tag="psp")
            for oh in range(2):
                nc.tensor.matmul(qp[:, oh, :],
                                 lhsT=wq8[:, :, oh * 128:(oh + 1) * 128],
                                 rhs=x8[:, :, qs], start=True, stop=True,
                                 perf_mode=DR)
            nc.scalar.activation(out=q8[:, 0, qs], in_=qp[:, 0, :],
                                 func=AF.Identity, bias=cq_sb[0], scale=1.0 / 16.0)
            nc.vector.tensor_scalar(out=q8[:, 1, qs], in0=qp[:, 1, :],
                                    scalar1=1.0 / 16.0, scalar2=cq_sb[1],
                                    op0=AL.mult, op1=AL.add)

        # ---- t = A/16 * (16Wk)^T q8   [c, n] fp8, ct on dim1 ----
        t8 = big.tile([128, 2, NQ], fp8, name="t8")
        for tc in range(2):
            ts_ = slice(tc * 512, (tc + 1) * 512)
            tp = psp.tile([128, 2, 512], f32, name="tp", tag="psp")
            for ct in range(2):
                nc.tensor.matmul(tp[:, ct, :],
                                 lhsT=wk8_sb[:, :, ct * 128:(ct + 1) * 128],
                                 rhs=q8[:, :, ts_], start=True, stop=True,
                                 perf_mode=DR)
            nc.scalar.activation(out=t8[:, 0, ts_], in_=tp[:, 0, :],
                                 func=AF.Identity, scale=Ai16_sb[0])
            nc.vector.tensor_scalar_mul(t8[:, 1, ts_], tp[:, 1, :], Ai16_sb[1])

        cv_sb = bias_vec(wvt_sb, B_sb, bv_sb, "cv")
        bpe_sb = bias_vec(wpt_sb, cv_sb, bp_sb, "bpe")

        # residual+bias base: xqb = x_q + bpe (Pool; off critical path)
        xqb = [big.tile([128, NQ], f32, name=f"xqb{ot}") for ot in range(2)]

        # ---- attention: v interleaved with S/exp/EV pipeline ----
        v8 = big.tile([128, 32, C], fp8, name="v8")
        att8 = []
        vmt = 0

        def v_pair():
            nonlocal vmt
            if vmt >= 32:
                return
            for j in range(2):
                mt = vmt + j
                vp = pvv.tile([128, 512], f32, name="vp", tag="pvv")
                nc.tensor.matmul(vp[:, 0:C], lhsT=x8[:, :, mt * 128:(mt + 1) * 128],
                                 rhs=wv8, start=True, stop=True, perf_mode=DR)
                nc.vector.tensor_scalar_mul(v8[:, mt, :], vp[:, 0:C], 1.0 / 16.0)
            vmt += 2

        for nch in range(2):
            ns = slice(nch * 512, (nch + 1) * 512)
            ot_t = pot.tile([128, 2, 512], f32, name="ot_t", tag="pot")
            rp = prp.tile([128, 512], f32, name="rp", tag="prp")
            e8s = [None] * 16

            def s_pair(p):
                sp = psp.tile([128, 2, 512], f32, name="sp", tag="psp")
                for j in range(2):
                    mt = 2 * p + j
                    nc.tensor.matmul(sp[:, j, :],
                                     lhsT=x8[:, :, mt * 128:(mt + 1) * 128],
                                     rhs=t8[:, :, ns], start=True, stop=True,
                                     perf_mode=DR)
                e8 = epool.tile([128, 2, 512], fp8, name="e8", tag="e8")
                nc.scalar.activation(out=e8[:, :, :], in_=sp[:, :, :],
                                     func=AF.Exp, scale=1.0 / 16.0, bias=eshift)
                e8s[p] = e8

            def r_ev(p):
                e8 = e8s[p]
                nc.tensor.matmul(rp, lhsT=ones8, rhs=e8[:, :, :],
                                 start=(p == 0), stop=(p == 15), perf_mode=DR)
                for oh in range(2):
                    nc.tensor.matmul(ot_t[:, oh, :],
                                     lhsT=v8[:, 2 * p:2 * p + 2,
                                             oh * 128:(oh + 1) * 128],
                                     rhs=e8[:, :, :], start=(p == 0),
                                     stop=(p == 15), perf_mode=DR)

            for p in range(16):
                v_pair()
                s_pair(p)
                if p > 0:
                    r_ev(p - 1)
            if nch == 0:
                for ot in range(2):
                    nc.gpsimd.tensor_scalar_add(xqb[ot], xq_sb[ot], bpe_sb[ot])
            r_ev(15)

            # normalize + drain att (DVE oh0 / Pool oh1), then project
            rb = work.tile([128, 512], f32, name="rb", tag="rb")
            nc.vector.reciprocal(out=rb, in_=rp)
            a8 = attp.tile([128, 2, 512], fp8, name="a8")
            nc.vector.scalar_tensor_tensor(out=a8[:, 0, :], in0=ot_t[:, 0, :],
                                           scalar=32.0, in1=rb,
                                           op0=AL.mult, op1=AL.mult)
            nc.vector.scalar_tensor_tensor(out=a8[:, 1, :], in0=ot_t[:, 1, :],
                                           scalar=32.0, in1=rb,
                                           op0=AL.mult, op1=AL.mult)
            att8.append(a8)

            for oh in range(2):
                pp = prp.tile([128, 512], f32, name="pp", tag="prp")
                nc.tensor.matmul(pp, lhsT=wp8_sb[:, :, oh * 128:(oh + 1) * 128],
                                 rhs=a8[:, :, :], start=True, stop=True,
                                 perf_mode=DR)
                ot_sb = work.tile([128, 512], f32, name="ot_sb", tag="ot_sb")
                nc.vector.scalar_tensor_tensor(out=ot_sb, in0=pp,
                                               scalar=1.0 / 512.0,
                                               in1=xqb[oh][:, ns],
                                               op0=AL.mult, op1=AL.add)
                nc.sync.dma_start(out=out_d[oh * 128:(oh + 1) * 128, ns], in_=ot_sb)

    nc.compile()
    return nc


def _get_nc():
    key = "nc"
    if key not in _CACHE:
        _CACHE[key] = _build_nc()
    return _CACHE[key]


def _host_inputs(x, gamma, beta, Wq, bq, Wk, bk, Wv, bv, Wp, bp):
    import ml_dtypes
    e4 = ml_dtypes.float8_e4m3

    x = np.asarray(x, np.float32)
    xf = np.ascontiguousarray(x.reshape(2, C, N))
    gamma = np.asarray(gamma, np.float32).reshape(C, 1)
    beta = np.asarray(beta, np.float32).reshape(C, 1)
    Wq = np.asarray(Wq, np.float32)
    Wk = np.asarray(Wk, np.float32)
    Wv = np.asarray(Wv, np.float32)
    Wp = np.asarray(Wp, np.float32)
    bq = np.asarray(bq, np.float32).reshape(C, 1)
    bv = np.asarray(bv, np.float32).reshape(C, 1)
    bp = np.asarray(bp, np.float32).reshape(C, 1)
    gmask = np.zeros((C, GROUPS), np.float32)
    gmask[np.arange(C), np.arange(C) // GSIZE] = 1.0
    gmask8 = np.ascontiguousarray(gmask.T)
    wall = np.ascontiguousarray(
        np.hstack([Wq.T, Wv.T, Wp.T]).astype(np.float32))
    small = np.ascontiguousarray(np.hstack([gamma, beta, bq, bv, bp, gmask]))

    # [o, c] -> [128, 2, c] with o = i*128+p on (p, i); flattened to [128, 2c]
    def pack8(m):
        m = np.ascontiguousarray((16.0 * m).astype(np.float32))
        m3 = m.reshape(2, 128, m.shape[1]).transpose(1, 0, 2)
        return np.ascontiguousarray(m3.reshape(128, -1)).astype(e4)

    wk8 = pack8(Wk)        # lhsT for t: contraction over o
    wp8 = pack8(Wp.T)      # lhsT for proj: contraction over c

    in_maps = []
    for core in range(8):
        b, j = divmod(core, 4)
        xrot = np.ascontiguousarray(np.roll(xf[b], -j * NQ, axis=1))
        x8 = xrot.reshape(2, 128, N).transpose(1, 0, 2)
        x8 = np.ascontiguousarray(x8.reshape(128, 2 * N)).astype(e4)
        in_maps.append({
            "x8": x8,
            "xq": np.ascontiguousarray(xrot[:, :NQ]),
            "wall": wall, "wk8": wk8, "wp8": wp8,
            "small": small, "gmask8": gmask8,
        })
    return in_maps


def kernel(x, gamma, beta, Wq, bq, Wk, bk, Wv, bv, Wp, bp):
    from concourse.bass_utils import run_bass_kernel_spmd
    global LAST_RESULTS

    orig_shape = np.asarray(x).shape
    in_maps = _host_inputs(x, gamma, beta, Wq, bq, Wk, bk, Wv, bv, Wp, bp)
    nc = _get_nc()

    trace = os.environ.get("BASSK_TRACE", "0") == "1"
    res = run_bass_kernel_spmd(nc, in_maps, core_ids=list(range(8)), trace=trace)
    LAST_RESULTS = res

    out = np.empty((2, C, N), np.float32)
    for core in range(8):
        b, j = divmod(core, 4)
        out[b][:, j * NQ:(j + 1) * NQ] = res.results[core]["out"]
    return out.reshape(orig_shape)


# revision 19
# speedup vs baseline: 1.9270x; 1.0448x over previous
"""AttentionBlock3D (GroupNorm + single-head self-attention + residual) on 8 TRN2 cores.

Sharding: core = (batch b in {0,1}) x (1024-row slice of the 4096 attention rows).
Each core redundantly computes its batch's GroupNorm stats and full V (cheap),
and attention + output projection for its own 1024 query rows. No collectives.
The host ROTATES each core's x copy so its query rows are always columns
0..1024 (attention is permutation-invariant over keys).

fp8 (e4m3) + DoubleRow PE mode: every 256-deep contraction runs as ONE
matmul over [128, 2, *] operand layouts at 0.5 cycles/row (4x the f32r
baseline). The k/q distinction is folded away:
  hn = A*x + B per channel, A = gamma*rsqrt(var_g+eps), B = beta - mean_g*A
  q  = (16*A.Wq)^T x8 / 16 + cq          cq = Wq B + bq
  t  = A/16 * (16*Wk)^T q8  (== Wk'^T q, the k-side collapsed into t)
  S^T[m,n] = sum_c x8[c,m] t8[c,n]       (k bias cancels in softmax)
  e  = exp(S/16 - 2.5)   (shift cancels in normalization; keeps e in fp8 range)
  r  = ones^T e;  v = (A.Wv)^T x8;  att = 32 * (E^T V) / r
  out = x_q + (bp + Wp cv) + (16Wp^T att)/512,  cv = Wv B + bv
GroupNorm stats come from a stride-4 subsample of the fp8 x copy (32k samples
per group; ~0.4% stat noise, far under tolerance). rsqrt is computed on DVE
with the bit-trick + 2 Newton iterations so ACT only ever needs the Exp table.
"""

import os
import numpy as np
from contextlib import ExitStack

C = 256          # channels
N = 4096         # spatial positions (16*16*16)
NQ = 1024        # query rows per core
GROUPS = 8
GSIZE = C // GROUPS
EPS = 1e-5
ESHIFT = 4.0     # exp(S/16 - ESHIFT): cancels in softmax, keeps e <= ~50 in fp8

_CACHE = {}
LAST_RESULTS = None  # test harness can inspect trace results


def _build_nc():
    import concourse.bacc as bacc
    import concourse.tile as tile
    from concourse import mybir

    f32 = mybir.dt.float32
    bf16 = mybir.dt.bfloat16
    fp8 = mybir.dt.float8e4
    i32 = mybir.dt.int32
    AF = mybir.ActivationFunctionType
    AL = mybir.AluOpType
    DR = mybir.MatmulPerfMode.DoubleRow

    nc = bacc.Bacc("TRN2", target_bir_lowering=False, debug=False,
                   enable_asserts=False)

    # ---- DRAM I/O (per-core) ----
    x8_d = nc.dram_tensor("x8", [128, 2 * N], fp8, kind="ExternalInput").ap()
    xq_d = nc.dram_tensor("xq", [128, 2 * NQ], f32, kind="ExternalInput").ap()
    wall_d = nc.dram_tensor("wall", [C, 3 * C], bf16, kind="ExternalInput").ap()
    wk8_d = nc.dram_tensor("wk8", [128, 2 * C], fp8, kind="ExternalInput").ap()
    wp8_d = nc.dram_tensor("wp8", [128, 2 * C], fp8, kind="ExternalInput").ap()
    small_d = nc.dram_tensor("small", [C, 5 + GROUPS], f32, kind="ExternalInput").ap()
    gmask8_d = nc.dram_tensor("gmask8", [GROUPS, C], f32, kind="ExternalInput").ap()
    out_d = nc.dram_tensor("out", [C, NQ], f32, kind="ExternalOutput").ap()

    with tile.TileContext(nc) as tc, ExitStack() as ctx:
        big = ctx.enter_context(tc.tile_pool(name="big", bufs=1))
        consts = ctx.enter_context(tc.tile_pool(name="consts", bufs=1))
        work = ctx.enter_context(tc.tile_pool(name="work", bufs=4))
        epool = ctx.enter_context(tc.tile_pool(name="epool", bufs=6))
        attp = ctx.enter_context(tc.tile_pool(name="attp", bufs=2))
        # PSUM: psp 2x[128,2,512]=4 banks, pvv 1, pot 2, prp 1 -> 8 banks
        psp = ctx.enter_context(tc.tile_pool(name="psp", bufs=2, space="PSUM"))
        pvv = ctx.enter_context(tc.tile_pool(name="pvv", bufs=1, space="PSUM"))
        pot = ctx.enter_context(tc.tile_pool(name="pot", bufs=1, space="PSUM"))
        prp = ctx.enter_context(tc.tile_pool(name="prp", bufs=1, space="PSUM"))

        # ---- consts + Exp act-table preload at t~0 (ACT's first instr) ----
        dume = consts.tile([1, 2], f32)
        nc.vector.memset(dume, 0.0)
        dume2 = consts.tile([1, 2], f32)
        nc.scalar.activation(out=dume2, in_=dume, func=AF.Exp, scale=1.0)
        ones8 = consts.tile([128, 2, 128], fp8)
        nc.vector.memset(ones8, 1.0)
        magic = consts.tile([GROUPS, 1], i32)
        nc.vector.memset(magic, 0x5F3759DF)
        sh1 = consts.tile([GROUPS, 1], i32)
        nc.vector.memset(sh1, 1)
        eshift = consts.tile([128, 1], f32)
        nc.vector.memset(eshift, -ESHIFT)

        # ---- DMAs (SP program order == priority) ----
        x8 = big.tile([128, 2, N], fp8, name="x8")
        for k in range(4):
            ct, h = divmod(k, 2)
            nc.sync.dma_start(out=x8[:, ct, h * 2048:(h + 1) * 2048],
                              in_=x8_d[:, k * 2048:(k + 1) * 2048])
        small_sb = []
        for ct in range(2):
            cs = slice(ct * 128, (ct + 1) * 128)
            t = consts.tile([128, 5 + GROUPS], f32, name=f"small_sb{ct}")
            nc.sync.dma_start(out=t, in_=small_d[cs, :])
            small_sb.append(t)
        gmask8_sb = consts.tile([GROUPS, C], f32)
        nc.sync.dma_start(out=gmask8_sb, in_=gmask8_d)
        wall_sb = []
        for ct in range(2):
            cs = slice(ct * 128, (ct + 1) * 128)
            t = consts.tile([128, 3 * C], bf16, name=f"wall_sb{ct}")
            nc.sync.dma_start(out=t, in_=wall_d[cs, :])
            wall_sb.append(t)
        wk8_sb = consts.tile([128, 2, C], fp8)
        nc.sync.dma_start(out=wk8_sb[:, :, :].bitcast(fp8), in_=wk8_d)
        xq_sb = big.tile([128, 2, NQ], f32, name="xq")
        nc.sync.dma_start(out=xq_sb[:, :, :].bitcast(f32), in_=xq_d)
        wp8_sb = consts.tile([128, 2, C], fp8)
        nc.sync.dma_start(out=wp8_sb[:, :, :].bitcast(fp8), in_=wp8_d)

        gamma_sb = [t[:, 0:1] for t in small_sb]
        beta_sb = [t[:, 1:2] for t in small_sb]
        bq_sb = [t[:, 2:3] for t in small_sb]
        bv_sb = [t[:, 3:4] for t in small_sb]
        bp_sb = [t[:, 4:5] for t in small_sb]
        gmask_sb = [t[:, 5:5 + GROUPS] for t in small_sb]
        wqt_sb = [t[:, 0 * C:1 * C] for t in wall_sb]
        wvt_sb = [t[:, 1 * C:2 * C] for t in wall_sb]
        wpt_sb = [t[:, 2 * C:3 * C] for t in wall_sb]

        # ---- GroupNorm stats from stride-4 subsample (DVE) ----
        gp = prp.tile([GROUPS, 2], f32, tag="prp")
        for ct in range(2):
            stats = work.tile([128, 2, 6], f32, name="stats", tag="stats")
            for h in range(2):
                nc.vector.bn_stats(out=stats[:, h, :],
                                   in_=x8[:, ct, h * 2048:(h + 1) * 2048:4])
            stile = work.tile([128, 2], f32, name="stile", tag="stile")
            msq = work.tile([128, 1], f32, name="msq", tag="msq")
            nc.vector.bn_aggr(out=stile, in_=stats)
            nc.vector.tensor_mul(msq, stile[:, 0:1], stile[:, 0:1])
            nc.vector.tensor_add(stile[:, 1:2], stile[:, 1:2], msq)
            nc.tensor.matmul(gp, lhsT=gmask_sb[ct], rhs=stile,
                             start=(ct == 0), stop=(ct == 1))

        # group mean / E[x^2] -> var -> rsqrt: bit-trick + 2 Newton iters
        # (fused: gvhn = -0.5*(var+eps) as a per-partition scalar AP)
        gms = work.tile([GROUPS, 2], f32, name="gms")
        gv = work.tile([GROUPS, 1], f32, name="gv")
        gvhn = work.tile([GROUPS, 1], f32, name="gvhn")
        gi = work.tile([GROUPS, 1], i32, name="gi")
        gy = work.tile([GROUPS, 1], f32, name="gy")
        nt = work.tile([GROUPS, 1], f32, name="nt")
        gsb = work.tile([GROUPS, 2], f32, name="gsb")
        nc.vector.tensor_scalar_mul(gms, gp, 1.0 / GSIZE)
        nc.vector.tensor_mul(gv, gms[:, 0:1], gms[:, 0:1])
        nc.vector.tensor_sub(gv, gms[:, 1:2], gv)
        nc.vector.tensor_scalar(out=gvhn, in0=gv, scalar1=-0.5,
                                scalar2=-0.5 * EPS, op0=AL.mult, op1=AL.add)
        nc.vector.tensor_scalar(out=gv, in0=gv, scalar1=EPS, scalar2=None,
                                op0=AL.add)
        nc.vector.tensor_scalar(out=gi, in0=gv.bitcast(i32), scalar1=sh1,
                                scalar2=None, op0=AL.logical_shift_right)
        nc.vector.tensor_sub(gy.bitcast(i32), magic, gi)
        for _ in range(2):
            nc.vector.tensor_mul(nt, gy, gy)
            nc.vector.tensor_scalar(out=nt, in0=nt, scalar1=gvhn, scalar2=1.5,
                                    op0=AL.mult, op1=AL.add)
            nc.vector.tensor_mul(gy, gy, nt)
        nc.vector.tensor_copy(gsb[:, 0:1], gms[:, 0:1])
        nc.vector.tensor_copy(gsb[:, 1:2], gy)

        # broadcast group stats to channels; A, B(bf16), A*16, A/16
        A_sb, A16_sb, Ai16_sb, B_sb = [], [], [], []
        for ct in range(2):
            gbp = prp.tile([128, 2], f32, name="gbp", tag="prp")
            nc.tensor.matmul(gbp, lhsT=gmask8_sb[:, ct * 128:(ct + 1) * 128],
                             rhs=gsb, start=True, stop=True)
            At = consts.tile([128, 1], f32, name=f"A_sb{ct}")
            A16 = consts.tile([128, 1], f32, name=f"A16_sb{ct}")
            Ai16 = consts.tile([128, 1], f32, name=f"Ai16_sb{ct}")
            Bt = consts.tile([128, 1], bf16, name=f"B_sb{ct}")
            nc.vector.tensor_mul(At, gamma_sb[ct], gbp[:, 1:2])
            nc.vector.scalar_tensor_tensor(out=Bt, in0=gbp[:, 0:1], scalar=-1.0,
                                           in1=At, op0=AL.mult, op1=AL.mult)
            nc.vector.tensor_add(Bt, Bt, beta_sb[ct])
            nc.vector.tensor_scalar_mul(A16, At, 16.0)
            nc.vector.tensor_scalar_mul(Ai16, At, 1.0 / 16.0)
            A_sb.append(At); A16_sb.append(A16)
            Ai16_sb.append(Ai16); B_sb.append(Bt)

        # fold 16*A into Wq / Wv, quantize to fp8 (DVE)
        wq8 = consts.tile([128, 2, C], fp8, name="wq8")
        wv8 = consts.tile([128, 2, C], fp8, name="wv8")
        for ct in range(2):
            nc.vector.tensor_scalar_mul(wq8[:, ct, :], wqt_sb[ct], A16_sb[ct])
        for ct in range(2):
            nc.vector.tensor_scalar_mul(wv8[:, ct, :], wvt_sb[ct], A16_sb[ct])

        # ---- bias vectors cq, cv, bpe (tiny bf16 PE matmuls) ----
        def bias_vec(wt_sb, rhs_tiles, badd, nm, dt=f32):
            outs = []
            for oh in range(2):
                p = prp.tile([128, 1], f32, name=f"{nm}p", tag="prp")
                for ct in range(2):
                    nc.tensor.matmul(p, lhsT=wt_sb[ct][:, oh * 128:(oh + 1) * 128],
                                     rhs=rhs_tiles[ct], start=(ct == 0), stop=(ct == 1))
                t = consts.tile([128, 1], dt, name=f"{nm}{oh}")
                nc.vector.tensor_scalar_add(t, p, badd[oh])
                outs.append(t)
            return outs

        cq_sb = bias_vec(wqt_sb, B_sb, bq_sb, "cq")

        # ---- q = (16A.Wq)^T x8 / 16 + cq   [o, n] fp8, oh on dim1 ----
        q8 = big.tile([128, 2, NQ], fp8, name="q8")
        for qc in range(2):
            qs = slice(qc * 512, (qc + 1) * 512)
            qp = psp.tile([128, 2, 512], f32, name="qp", tag="psp")
            for oh in range(2):
                nc.tensor.matmul(qp[:, oh, :],
                                 lhsT=wq8[:, :, oh * 128:(oh + 1) * 128],
                                 rhs=x8[:, :, qs], start=True, stop=True,
                                 perf_mode=DR)
            nc.scalar.activation(out=q8[:, 0, qs], in_=qp[:, 0, :],
                                 func=AF.Identity, bias=cq_sb[0], scale=1.0 / 16.0)
            nc.vector.tensor_scalar(out=q8[:, 1, qs], in0=qp[:, 1, :],
                                    scalar1=1.0 / 16.0, scalar2=cq_sb[1],
                                    op0=AL.mult, op1=AL.add)

        # ---- t = A/16 * (16Wk)^T q8   [c, n] fp8, ct on dim1 ----
        t8 = big.tile([128, 2, NQ], fp8, name="t8")
        for tc in range(2):
            ts_ = slice(tc * 512, (tc + 1) * 512)
            tp = psp.tile([128, 2, 512], f32, name="tp", tag="psp")
            for ct in range(2):
                nc.tensor.matmul(tp[:, ct, :],
                                 lhsT=wk8_sb[:, :, ct * 128:(ct + 1) * 128],
                                 rhs=q8[:, :, ts_], start=True, stop=True,
                                 perf_mode=DR)
            nc.scalar.activation(out=t8[:, 0, ts_], in_=tp[:, 0, :],
                                 func=AF.Identity, scale=Ai16_sb[0])
            nc.vector.tensor_scalar_mul(t8[:, 1, ts_], tp[:, 1, :], Ai16_sb[1])

        cv_sb = bias_vec(wvt_sb, B_sb, bv_sb, "cv", dt=bf16)
        bpe_sb = bias_vec(wpt_sb, cv_sb, bp_sb, "bpe")

        # residual+bias base: xqb = x_q + bpe (Pool; off critical path)
        xqb = big.tile([128, 2, NQ], f32, name="xqb")

        # ---- attention: v interleaved with S/exp/EV pipeline ----
        v8 = big.tile([128, 32, C], fp8, name="v8")
        att8 = []
        vmt = 0

        def v_pair():
            nonlocal vmt
            if vmt >= 32:
                return
            for j in range(2):
                mt = vmt + j
                vp = pvv.tile([128, 512], f32, name="vp", tag="pvv")
                nc.tensor.matmul(vp[:, 0:C], lhsT=x8[:, :, mt * 128:(mt + 1) * 128],
                                 rhs=wv8, start=True, stop=True, perf_mode=DR)
                nc.vector.tensor_scalar_mul(v8[:, mt, :], vp[:, 0:C], 1.0 / 16.0)
            vmt += 2

        for nch in range(2):
            ns = slice(nch * 512, (nch + 1) * 512)
            ot_t = pot.tile([128, 2, 512], f32, name="ot_t", tag="pot")
            rp = prp.tile([128, 512], f32, name="rp", tag="prp")
            e8s = [None] * 16

            def s_pair(p):
                sp = psp.tile([128, 2, 512], f32, name="sp", tag="psp")
                for j in range(2):
                    mt = 2 * p + j
                    nc.tensor.matmul(sp[:, j, :],
                                     lhsT=x8[:, :, mt * 128:(mt + 1) * 128],
                                     rhs=t8[:, :, ns], start=True, stop=True,
                                     perf_mode=DR)
                e8 = epool.tile([128, 2, 512], fp8, name="e8", tag="e8")
                nc.scalar.activation(out=e8[:, :, :], in_=sp[:, :, :],
                                     func=AF.Exp, scale=1.0 / 16.0, bias=eshift)
                e8s[p] = e8

            def r_ev(p):
                e8 = e8s[p]
                nc.tensor.matmul(rp, lhsT=ones8, rhs=e8[:, :, :],
                                 start=(p == 0), stop=(p == 15), perf_mode=DR)
                for oh in range(2):
                    nc.tensor.matmul(ot_t[:, oh, :],
                                     lhsT=v8[:, 2 * p:2 * p + 2,
                                             oh * 128:(oh + 1) * 128],
                                     rhs=e8[:, :, :], start=(p == 0),
                                     stop=(p == 15), perf_mode=DR)

            for p in range(16):
                s_pair(p)
                v_pair()
                if p > 0:
                    r_ev(p - 1)
            if nch == 0:
                for ot in range(2):
                    nc.gpsimd.tensor_scalar_add(xqb[:, ot, :], xq_sb[:, ot, :],
                                                bpe_sb[ot])
            r_ev(15)

            # normalize + drain att (DVE), then project
            rb = work.tile([128, 512], f32, name="rb", tag="rb")
            nc.vector.reciprocal(out=rb, in_=rp)
            a8 = attp.tile([128, 2, 512], fp8, name="a8")
            nc.vector.scalar_tensor_tensor(out=a8[:, 0, :], in0=ot_t[:, 0, :],
                                           scalar=32.0, in1=rb,
                                           op0=AL.mult, op1=AL.mult)
            nc.vector.scalar_tensor_tensor(out=a8[:, 1, :], in0=ot_t[:, 1, :],
                                           scalar=32.0, in1=rb,
                                           op0=AL.mult, op1=AL.mult)
            att8.append(a8)

            # halves on the last chunk: earlier DMA dispatch shortens the tail
            nh = 2 if nch == 1 else 1
            for oh in range(2):
                pp = pvv.tile([128, 512], f32, name="pp", tag="pvv")
                nc.tensor.matmul(pp, lhsT=wp8_sb[:, :, oh * 128:(oh + 1) * 128],
                                 rhs=a8[:, :, :], start=True, stop=True,
                                 perf_mode=DR)
                for h in range(nh):
                    hs = slice(h * (512 // nh), (h + 1) * (512 // nh))
                    ds = slice(nch * 512 + h * (512 // nh),
                               nch * 512 + (h + 1) * (512 // nh))
                    ot_sb = work.tile([128, 512 // nh], f32, name="ot_sb",
                                      tag="ot_sb")
                    nc.vector.scalar_tensor_tensor(out=ot_sb, in0=pp[:, hs],
                                                   scalar=1.0 / 512.0,
                                                   in1=xqb[:, oh, ds],
                                                   op0=AL.mult, op1=AL.add)
                    nc.sync.dma_start(out=out_d[oh * 128:(oh + 1) * 128, ds],
                                      in_=ot_sb)

    nc.compile()
    return nc


def _get_nc():
    key = "nc"
    if key not in _CACHE:
        _CACHE[key] = _build_nc()
    return _CACHE[key]


def _host_inputs(x, gamma, beta, Wq, bq, Wk, bk, Wv, bv, Wp, bp):
    import ml_dtypes
    e4 = ml_dtypes.float8_e4m3

    x = np.asarray(x, np.float32)
    xf = np.ascontiguousarray(x.reshape(2, C, N))
    gamma = np.asarray(gamma, np.float32).reshape(C, 1)
    beta = np.asarray(beta, np.float32).reshape(C, 1)
    Wq = np.asarray(Wq, np.float32)
    Wk = np.asarray(Wk, np.float32)
    Wv = np.asarray(Wv, np.float32)
    Wp = np.asarray(Wp, np.float32)
    bq = np.asarray(bq, np.float32).reshape(C, 1)
    bv = np.asarray(bv, np.float32).reshape(C, 1)
    bp = np.asarray(bp, np.float32).reshape(C, 1)
    gmask = np.zeros((C, GROUPS), np.float32)
    gmask[np.arange(C), np.arange(C) // GSIZE] = 1.0
    gmask8 = np.ascontiguousarray(gmask.T)
    wall = np.ascontiguousarray(
        np.hstack([Wq.T, Wv.T, Wp.T]).astype(ml_dtypes.bfloat16))
    small = np.ascontiguousarray(np.hstack([gamma, beta, bq, bv, bp, gmask]))

    # [o, c] -> [128, 2, c] with o = i*128+p on (p, i); flattened to [128, 2c]
    def pack8(m):
        m = np.ascontiguousarray((16.0 * m).astype(np.float32))
        m3 = m.reshape(2, 128, m.shape[1]).transpose(1, 0, 2)
        return np.ascontiguousarray(m3.reshape(128, -1)).astype(e4)

    wk8 = pack8(Wk)        # lhsT for t: contraction over o
    wp8 = pack8(Wp.T)      # lhsT for proj: contraction over c

    in_maps = []
    for core in range(8):
        b, j = divmod(core, 4)
        xrot = np.ascontiguousarray(np.roll(xf[b], -j * NQ, axis=1))
        x8 = xrot.reshape(2, 128, N).transpose(1, 0, 2)
        x8 = np.ascontiguousarray(x8.reshape(128, 2 * N)).astype(e4)
        xq = xrot[:, :NQ].reshape(2, 128, NQ).transpose(1, 0, 2)
        xq = np.ascontiguousarray(xq.reshape(128, 2 * NQ))
        in_maps.append({
            "x8": x8, "xq": xq,
            "wall": wall, "wk8": wk8, "wp8": wp8,
            "small": small, "gmask8": gmask8,
        })
    return in_maps


def kernel(x, gamma, beta, Wq, bq, Wk, bk, Wv, bv, Wp, bp):
    from concourse.bass_utils import run_bass_kernel_spmd
    global LAST_RESULTS

    orig_shape = np.asarray(x).shape
    in_maps = _host_inputs(x, gamma, beta, Wq, bq, Wk, bk, Wv, bv, Wp, bp)
    nc = _get_nc()

    trace = os.environ.get("BASSK_TRACE", "0") == "1"
    res = run_bass_kernel_spmd(nc, in_maps, core_ids=list(range(8)), trace=trace)
    LAST_RESULTS = res

    out = np.empty((2, C, N), np.float32)
    for core in range(8):
        b, j = divmod(core, 4)
        out[b][:, j * NQ:(j + 1) * NQ] = res.results[core]["out"]
    return out.reshape(orig_shape)
